# revision 1
# baseline (speedup 1.0000x reference)
"""Gaussian falloff vortex-velocity kernel for Trainium2 (8 NeuronCores).

Math: out[b,h,w,:] = sum_n tau_n * exp(-r2/sig_n^2) / sqrt(r2) * (d2, -d1)
with d1 = py - y_n, d2 = px - x_n, r2 = d1^2 + d2^2.

Device algorithm (per core, H sharded 8 ways):
  1. PE computes t2' = a_n*(r2 + eps_n) for 128 particles x 512 points per
     matmul, where a_n = 2/sig_n^2, via a K=31 contraction of triple-bf16-split
     terms: a*py^2 - 2a*y*py + a*y^2 + a*px^2 - 2a*x*px + a*x^2 + a*eps.
     Rows are ordered so partial sums telescope near zero for close pairs,
     keeping fp32 accumulation error ~1e-6 in r2 units.
  2. ACT: lt = Ln(t2')                 (PSUM -> SBUF, fp32)
  3. DVE: w  = -t2' - lt               (one scalar_tensor_tensor, fp32)
  4. ACT: g  = Exp(0.5*w)              (-> bf16)  [= exp(-t2'/2)/sqrt(t2')]
  5. PE: S_r = sum_n w_rn * g_n  for r in {0,1,2} with hi/lo-split bf16
     weights {tau*q, tau*x*q, tau*y*q}, q = exp(a*eps/2)*sqrt(a).
  6. DVE: u = px*S0 - S1, v = S2 - py*S0 (after a DRAM relayout round-trip).
Ln and Exp share one ACT table set (natural_log_exp_and_others).
"""

import sys

import numpy as np

B, H, W, N = 2, 256, 256, 512
NCORES = 8
HPC = H // NCORES          # 32 rows per core
PPB = HPC * W              # 8192 points per batch per core
NT = PPB // 512            # 16 point-tiles of 512 per batch
NK = N // 128              # 4 particle blocks
KROWS = 31
EPS0, EPS1 = 2e-6, 1.5e-6

_cache = {}


def _bass_modules():
    if "/opt/trn_rl_repo" not in sys.path:
        sys.path.insert(0, "/opt/trn_rl_repo")
    import concourse.bass as bass
    import concourse.mybir as mybir
    import concourse.tile as tile
    from concourse import bacc
    from concourse.bass_utils import run_bass_kernel_spmd

    return bass, mybir, tile, run_bass_kernel_spmd, bacc


def _pin_act_table_set():
    """Make the table-load pass satisfy Ln/Exp only from the combined set so
    alternating Ln/Exp instructions never thrash ACT table loads."""
    import concourse.bacc as bacc_mod
    import concourse.mybir as mybir

    if getattr(bacc_mod, "_act_tables_pinned", False):
        return
    orig = bacc_mod.get_activation_tables
    ln_exp = {mybir.ActivationFunctionType.Ln, mybir.ActivationFunctionType.Exp}

    def patched(arch):
        tables = orig(arch)
        keep = "natural_log_exp_and_others"
        if keep not in tables:
            return tables
        return {
            name: (funcs if name == keep else (funcs - ln_exp))
            for name, funcs in tables.items()
        }

    bacc_mod.get_activation_tables = patched
    bacc_mod._act_tables_pinned = True


def _build_nc():
    bass, mybir, tile, _, bacc = _bass_modules()
    _pin_act_table_set()
    f32 = mybir.dt.float32
    bf16 = mybir.dt.bfloat16
    AF = mybir.ActivationFunctionType
    ALU = mybir.AluOpType

    nc = bacc.Bacc(None)
    rhs_d = nc.declare_dram_parameter("rhs", [B, 16, KROWS, PPB // 16], bf16, isOutput=False)
    lhst_d = nc.declare_dram_parameter("lhst", [B, KROWS, N], bf16, isOutput=False)
    wm_d = nc.declare_dram_parameter("wmat", [128, B * NK * 6], bf16, isOutput=False)
    ptsf_d = nc.declare_dram_parameter("ptsf", [B, 2, 128, PPB // 128], f32, isOutput=False)
    out_d = nc.declare_dram_parameter("out", [B, 2, 128, PPB // 128], f32, isOutput=True)

    with tile.TileContext(nc) as tc:
        with (
            tc.tile_pool(name="const", bufs=1) as cpool,
            tc.tile_pool(name="lts", bufs=4) as ltpool,
            tc.tile_pool(name="wg", bufs=2) as wgpool,
            tc.tile_pool(name="stg", bufs=2) as stgpool,
            tc.tile_pool(name="fin", bufs=2) as fin,
            tc.tile_pool(name="r2p", bufs=3, space=bass.MemorySpace.PSUM) as r2pool,
            tc.tile_pool(name="sap", bufs=2, space=bass.MemorySpace.PSUM) as spool,
            tc.tile_pool(name="dscratch", bufs=1, space="DRAM") as dpool,
        ):
            rhs_sb, lhs_sb = [], []
            for b in range(B):
                t = cpool.tile([KROWS, PPB], bf16, tag=f"rhs{b}")
                rhs_sb.append(t)
                t2 = cpool.tile([KROWS, N], bf16, tag=f"lhs{b}")
                lhs_sb.append(t2)
            # first compute chunk unblocks ASAP, rest follow
            CW = PPB // 16
            nc.sync.dma_start(rhs_sb[0][:, 0:CW], rhs_d[0, 0])
            nc.sync.dma_start(lhs_sb[0][:], lhst_d[0])
            wm = cpool.tile([128, B * NK * 6], bf16, tag="wm")
            nc.sync.dma_start(wm[:], wm_d[:])
            for c in range(1, 16):
                nc.sync.dma_start(
                    rhs_sb[0][:, c * CW : (c + 1) * CW], rhs_d[0, c]
                )
            nc.sync.dma_start(lhs_sb[1][:], lhst_d[1])
            for c in range(16):
                nc.sync.dma_start(
                    rhs_sb[1][:, c * CW : (c + 1) * CW], rhs_d[1, c]
                )
            ptq = {}
            for b in range(B):
                for q in range(4):
                    pq = cpool.tile([32, 128], f32, tag=f"ptq{b}{q}", name=f"ptq{b}{q}")
                    nc.sync.dma_start(
                        pq[:, 0:64], ptsf_d[b, 0, q * 32 : (q + 1) * 32]
                    )
                    nc.sync.dma_start(
                        pq[:, 64:128], ptsf_d[b, 1, q * 32 : (q + 1) * 32]
                    )
                    ptq[(b, q)] = pq
            scratch = dpool.tile([B, 4, 6, PPB // 4], f32)
            srs = scratch[:].rearrange("b q six (p f) -> b q p six f", p=32)

            wt_t, g_t, sacc_t, stage_t = {}, {}, {}, {}

            def stage_a(gi, half):  # r2 matmuls + Ln + stt for tile gi, chunk half
                b, T = divmod(gi, NT)
                if half == 0:
                    wt_t[gi] = wgpool.tile(
                        [128, 2048], f32, tag="wt", name=f"wt{gi}", bufs=3
                    )
                wt = wt_t[gi]
                if T == 0 and half == 0:
                    stage_t[b] = stgpool.tile(
                        [6, PPB], f32, tag="sstage", name=f"sstage{b}"
                    )
                if True:
                    sl = slice(T * 512, (T + 1) * 512)
                    for p in (half,):
                        r2t = r2pool.tile([128, 1024], f32, tag="r2")
                        for hh in range(2):
                            k = 2 * p + hh
                            nc.tensor.matmul(
                                r2t[:, hh * 512 : (hh + 1) * 512],
                                lhs_sb[b][:, k * 128 : (k + 1) * 128],
                                rhs_sb[b][:, sl],
                                start=True,
                                stop=True,
                            )
                        lt = ltpool.tile([128, 1024], f32, tag="lt")
                        nc.scalar.activation(lt[:], r2t[:], AF.Ln)
                        nc.vector.scalar_tensor_tensor(
                            wt[:, p * 1024 : (p + 1) * 1024],
                            r2t[:],
                            -1.0,
                            lt[:],
                            ALU.mult,
                            ALU.subtract,
                        )

            def stage_b(gi):  # Exp for tile gi
                g = wgpool.tile([128, 2048], bf16, tag="g", bufs=3)
                g_t[gi] = g
                nc.scalar.activation(g[:], wt_t.pop(gi)[:], AF.Exp, scale=0.5)

            def stage_c(gi):  # S-matmuls + staging copy for tile gi
                b, T = divmod(gi, NT)
                g = g_t.pop(gi)
                sstage = stage_t[b]
                sl = slice(T * 512, (T + 1) * 512)
                sacc = spool.tile([6, 512], f32, tag="sacc")
                for k in range(NK):
                    c6 = (b * NK + k) * 6
                    gk = g[:, k * 512 : (k + 1) * 512]
                    nc.tensor.matmul(
                        sacc[:], wm[:, c6 : c6 + 6], gk,
                        start=(k == 0), stop=(k == NK - 1),
                    )
                nc.vector.tensor_copy(sstage[:, sl], sacc[:])
                if T % 4 == 3:
                    finish_quarter(b, T // 4)

            QW = PPB // 4  # points per quarter

            def finish_quarter(b, q):
                sstage = stage_t[b]
                qs = slice(q * QW, (q + 1) * QW)
                weng = nc.scalar if b == B - 1 and q == 3 else nc.sync
                weng.dma_start(scratch[b, q], sstage[:, qs])
                last = b == B - 1 and q == 3
                veng = nc.vector if last else nc.gpsimd
                sh = []
                for rr in range(6):
                    s = fin.tile([32, 64], f32, tag=f"s{rr}", name=f"s{rr}_{b}_{q}")
                    eng = nc.scalar if (last and rr % 2 == 1) else nc.sync
                    eng.dma_start(s[:], srs[b, q, :, rr])
                    sh.append(s[:])
                
                pyf = ptq[(b, q)][:, 0:64]
                pxf = ptq[(b, q)][:, 64:128]
                s0 = fin.tile([32, 64], f32, tag="s0t")
                veng.tensor_add(s0[:], sh[0], sh[3])
                s1 = fin.tile([32, 64], f32, tag="s1t")
                veng.tensor_add(s1[:], sh[1], sh[4])
                s2 = fin.tile([32, 64], f32, tag="s2t")
                veng.tensor_add(s2[:], sh[2], sh[5])
                tu = fin.tile([32, 64], f32, tag="tu")
                veng.tensor_mul(tu[:], pxf, s0[:])
                u = fin.tile([32, 64], f32, tag="u")
                veng.tensor_sub(u[:], tu[:], s1[:])
                tv = fin.tile([32, 64], f32, tag="tv")
                veng.tensor_mul(tv[:], pyf, s0[:])
                v = fin.tile([32, 64], f32, tag="v")
                veng.tensor_sub(v[:], s2[:], tv[:])
                (nc.scalar if last else nc.sync).dma_start(out_d[b, 0, q * 32 : (q + 1) * 32], u[:])
                nc.sync.dma_start(out_d[b, 1, q * 32 : (q + 1) * 32], v[:])

            NG = B * NT
            STEP = 0.004  # ms of logical time per pipeline iteration
            for gi in range(NG + 2):
                t_it = STEP * gi
                if gi < NG:
                    with tc.tile_wait_until(t_it):
                        stage_a(gi, 0)
                if 1 <= gi <= NG:
                    with tc.tile_wait_until(t_it + 0.001):
                        stage_b(gi - 1)
                if gi < NG:
                    with tc.tile_wait_until(t_it + 0.002):
                        stage_a(gi, 1)
                if 2 <= gi <= NG + 1:
                    with tc.tile_wait_until(t_it + 0.003):
                        stage_c(gi - 2)
    nc.compile()
    return nc


def _split3(a, bf):
    h = a.astype(bf)
    m = (a - h.astype(np.float64)).astype(bf)
    l = (a - h.astype(np.float64) - m.astype(np.float64)).astype(bf)
    return h, m, l


def _prep_inputs(vortex_feature, points):
    import ml_dtypes

    bf = ml_dtypes.bfloat16
    vf = np.asarray(vortex_feature, dtype=np.float64)
    pts_full = np.asarray(points, dtype=np.float64)
    y, x, tau = vf[:, :, 0], vf[:, :, 1], vf[:, :, 2]
    sig2 = vf[:, :, 3] ** 2
    a_n = 2.0 / sig2
    eps_n = EPS0 + EPS1 * (y * y + x * x)

    # lhsT rows [B, KROWS, N]: triple-split entries; order must match rhs rows.
    lhst = np.zeros((B, KROWS, N), dtype=bf)
    for b in range(B):
        A3 = _split3(a_n[b], bf)
        CY3 = _split3(-2.0 * a_n[b] * y[b], bf)
        CX3 = _split3(-2.0 * a_n[b] * x[b], bf)
        AYY3 = _split3(a_n[b] * y[b] * y[b], bf)
        AXX3 = _split3(a_n[b] * x[b] * x[b], bf)
        aeps = (a_n[b] * eps_n[b]).astype(bf)
        rows = []
        for (uh, um, ul) in (A3, CY3):
            rows += [uh, uh, um, uh, ul, um]
        rows += list(AYY3)
        for (uh, um, ul) in (A3, CX3):
            rows += [uh, uh, um, uh, ul, um]
        rows += list(AXX3)
        rows.append(aeps)
        lhst[b] = np.stack(rows, 0)

    # weights with eps correction, hi/lo split side by side: [128, B*NK*6]
    q = np.exp(0.5 * a_n * eps_n) * np.sqrt(a_n)
    wfull = np.stack([tau * q, tau * x * q, tau * y * q], axis=-1)  # [B, N, 3]
    whd = wfull.astype(bf)
    wld = (wfull - whd.astype(np.float64)).astype(bf)
    w6 = np.concatenate([whd, wld], axis=-1)  # [B, N, 6]
    wm = np.ascontiguousarray(
        w6.reshape(B, NK, 128, 6).transpose(2, 0, 1, 3).reshape(128, B * NK * 6)
    )

    in_maps = []
    for i in range(NCORES):
        slp = pts_full[:, i * HPC : (i + 1) * HPC].reshape(B, PPB, 2)
        pts = np.ascontiguousarray(slp.transpose(0, 2, 1))  # [B, 2, PPB]
        ptsf = np.ascontiguousarray(
            pts.reshape(B, 2, 128, PPB // 128), dtype=np.float32
        )
        rhs = np.zeros((B, KROWS, PPB), dtype=bf)
        for b in range(B):
            py, px = pts[b, 0], pts[b, 1]
            PYY3 = _split3(py * py, bf)
            PY3 = _split3(py, bf)
            PXX3 = _split3(px * px, bf)
            PX3 = _split3(px, bf)
            ones = np.ones(PPB, dtype=bf)
            rows = []
            for (wh_, wm_, wl_) in (PYY3, PY3):
                rows += [wh_, wm_, wh_, wl_, wh_, wm_]
            rows += [ones] * 3
            for (wh_, wm_, wl_) in (PXX3, PX3):
                rows += [wh_, wm_, wh_, wl_, wh_, wm_]
            rows += [ones] * 3
            rows.append(ones)
            rhs[b] = np.stack(rows, 0)
        rhs16 = np.ascontiguousarray(
            rhs.reshape(B, KROWS, 16, PPB // 16).transpose(0, 2, 1, 3)
        )
        in_maps.append({"rhs": rhs16, "lhst": lhst, "wmat": wm, "ptsf": ptsf})
    return in_maps


def _assemble(results):
    out = np.zeros((B, H, W, 2), dtype=np.float32)
    for i in range(NCORES):
        o = np.asarray(results[i]["out"])  # [B, 2, 128, PPB//128]
        o = o.reshape(B, 2, PPB).transpose(0, 2, 1).reshape(B, HPC, W, 2)
        out[:, i * HPC : (i + 1) * HPC] = o
    return out


def _run(vortex_feature, points, trace=False):
    _, _, _, run_bass_kernel_spmd, _b = _bass_modules()
    if "nc" not in _cache:
        _cache["nc"] = _build_nc()
    in_maps = _prep_inputs(vortex_feature, points)
    res = run_bass_kernel_spmd(
        _cache["nc"], in_maps, list(range(NCORES)), trace=trace
    )
    return _assemble(res.results), res


def kernel(vortex_feature, points):
    out, _ = _run(vortex_feature, points, trace=False)
    return out



# revision 4
# speedup vs baseline: 1.0542x; 1.0542x over previous
"""Gaussian falloff vortex-velocity kernel for Trainium2 (8 NeuronCores).

Math: out[b,h,w,:] = sum_n tau_n * exp(-r2/sig_n^2) / sqrt(r2) * (d2, -d1)
with d1 = py - y_n, d2 = px - x_n, r2 = d1^2 + d2^2.

Device algorithm (per core, H sharded 8 ways). Let v = a'*(r2+eps) with
a' = 1/sig^2, so the falloff g = exp(-v)/sqrt(v) (per-particle factors
q = exp(a'*eps)*sqrt(a') are folded into the S-weights).

  1. PE computes X = -v for 128 particles x 1024 points per half-tile via a
     K=31 triple-bf16-split contraction (negated, telescoped row order).
  2. Most halves (I-path): DVE computes w1 = -c*float(int32bits(X)) + K
     (fast-log: -0.5*ln(v) + 44-ish with a +-1.5% sawtooth, centered), then
     the PE accumulates w1 onto X via an identity matmul: X = w1 - v.
     ACT then applies one Exp (bias folds the 44): g = exp(-v)/sqrt(v).
  3. Some halves (A-path, exact): ACT Ln(-X) -> lt; DVE w = -0.5*lt + X;
     ACT Exp(w).  Ratio balances PE vs ACT load.
  4. PE S-matmuls: S_r(p) = sum_n w_rn * g_np for r in {tau, tau*x, tau*y}
     with single-fp16 weights, accumulated into 32-aligned partition slots
     (4 point-tiles per PSUM bank via tile_position).
  5. S tiles are copied PSUM->SBUF and DMAed out; the host computes
     u = px*S0 - S1, v = S2 - py*S0 and assembles the [B,H,W,2] output.

Ln and Exp share one ACT table set (natural_log_exp_and_others).
"""

import sys

import numpy as np

B, H, W, N = 2, 256, 256, 512
NCORES = 8
HPC = H // NCORES          # 32 rows per core
PPB = HPC * W              # 8192 points per batch per core
NT = PPB // 512            # 16 point-tiles of 512 per batch
NH = NT * 2                # 32 half-tiles (1024 points) per batch? no: halves of particle blocks
NK = N // 128              # 4 particle blocks
KROWS = 31
EPS0, EPS1 = 2e-6, 1.5e-6

LN2 = float(np.log(2.0))
CFAST = 0.5 * LN2 * 2.0**-23            # fast-log slope
KBIAS = 0.5 * LN2 * (127.0 - 0.0430357)  # sawtooth-centered constant (ACT bias)
K1 = -CFAST * (2.0**31) + KBIAS          # ts constant (sign-bit of -v absorbed)

# A-path (exact Ln) for halves where (u % APER) == APER-1
APER = 4

_cache = {}


def _bass_modules():
    if "/opt/trn_rl_repo" not in sys.path:
        sys.path.insert(0, "/opt/trn_rl_repo")
    import concourse.bass as bass
    import concourse.mybir as mybir
    import concourse.tile as tile
    from concourse import bacc
    from concourse.bass_utils import run_bass_kernel_spmd

    return bass, mybir, tile, run_bass_kernel_spmd, bacc


def _pin_act_table_set():
    """Make the table-load pass satisfy Ln/Exp only from the combined set so
    alternating Ln/Exp instructions never thrash ACT table loads."""
    import concourse.bacc as bacc_mod
    import concourse.mybir as mybir

    if getattr(bacc_mod, "_act_tables_pinned", False):
        return
    orig = bacc_mod.get_activation_tables
    ln_exp = {mybir.ActivationFunctionType.Ln, mybir.ActivationFunctionType.Exp}

    def patched(arch):
        tables = orig(arch)
        keep = "natural_log_exp_and_others"
        if keep not in tables:
            return tables
        return {
            name: (funcs if name == keep else (funcs - ln_exp))
            for name, funcs in tables.items()
        }

    bacc_mod.get_activation_tables = patched
    bacc_mod._act_tables_pinned = True


def _build_nc(step_ms=0.00145):
    bass, mybir, tile, _, bacc = _bass_modules()
    _pin_act_table_set()
    f32 = mybir.dt.float32
    i32 = mybir.dt.int32
    bf16 = mybir.dt.bfloat16
    fp16 = mybir.dt.float16
    AF = mybir.ActivationFunctionType
    ALU = mybir.AluOpType

    nc = bacc.Bacc(None)
    rhs_d = nc.declare_dram_parameter("rhs", [B, 16, KROWS, PPB // 16], bf16, isOutput=False)
    lhst_d = nc.declare_dram_parameter("lhst", [B, KROWS, N], bf16, isOutput=False)
    wq_d = nc.declare_dram_parameter("wq", [128, B * NK * 3], fp16, isOutput=False)
    eye_d = nc.declare_dram_parameter("eye", [128, 128], fp16, isOutput=False)
    bias_d = nc.declare_dram_parameter("biasv", [128, 2], f32, isOutput=False)
    out_d = nc.declare_dram_parameter("out", [B, NT // 4, 128, 512], f32, isOutput=True)

    NHALF = B * NT * 2  # total half-tiles: each (b, T) has 2 halves of 1024 pts? no:
    # Each (b,T) covers 512 points with 4 particle blocks; halves split blocks:
    # half h covers particle blocks k=2h,2h+1 -> X [128, 1024] = 2 blocks x 512 pts.

    with tile.TileContext(nc) as tc:
        with (
            tc.tile_pool(name="const", bufs=1) as cpool,
            tc.tile_pool(name="w1p", bufs=3) as w1pool,
            tc.tile_pool(name="ltp", bufs=3) as ltpool,
            tc.tile_pool(name="gp", bufs=3) as gpool,
            tc.tile_pool(name="stg", bufs=2) as stgpool,
            tc.tile_pool(name="xp", bufs=3, space=bass.MemorySpace.PSUM) as xpool,
            tc.tile_pool(name="sap", bufs=2, space=bass.MemorySpace.PSUM) as spool,
        ):
            rhs_sb, lhs_sb = [], []
            for b in range(B):
                t = cpool.tile([KROWS, PPB], bf16, tag=f"rhs{b}")
                rhs_sb.append(t)
                t2 = cpool.tile([KROWS, N], bf16, tag=f"lhs{b}")
                lhs_sb.append(t2)
            CW = PPB // 16
            nc.sync.dma_start(rhs_sb[0][:, 0:CW], rhs_d[0, 0])
            nc.sync.dma_start(lhs_sb[0][:], lhst_d[0])
            wq = cpool.tile([128, B * NK * 3], fp16, tag="wq")
            nc.sync.dma_start(wq[:], wq_d[:])
            eye = cpool.tile([128, 128], fp16, tag="eye")
            nc.sync.dma_start(eye[:], eye_d[:])
            biasv = cpool.tile([128, 2], f32, tag="biasv")
            nc.sync.dma_start(biasv[:], bias_d[:])
            for c in range(1, 16):
                nc.sync.dma_start(rhs_sb[0][:, c * CW : (c + 1) * CW], rhs_d[0, c])
            nc.sync.dma_start(lhs_sb[1][:], lhst_d[1])
            for c in range(16):
                nc.sync.dma_start(rhs_sb[1][:, c * CW : (c + 1) * CW], rhs_d[1, c])

            X_t, w_t, g_t, sacc_t = {}, {}, {}, {}

            def half_info(u):
                gi, h = divmod(u, 2)
                b, T = divmod(gi, NT)
                return b, T, h

            def stage1(u):  # v-matmuls -> X = -v
                b, T, h = half_info(u)
                apath = u % APER == APER - 1
                X = xpool.tile([128, 1024], f32, tag="X", name=f"X{u}")
                X_t[u] = X
                sl = slice(T * 512, (T + 1) * 512)
                for hh in range(2):
                    k = 2 * h + hh
                    nc.tensor.matmul(
                        X[:, hh * 512 : (hh + 1) * 512],
                        lhs_sb[b][:, k * 128 : (k + 1) * 128],
                        rhs_sb[b][:, sl],
                        start=True,
                        stop=apath,
                    )

            def stage2(u):  # fast-log + identity merge (I) or Ln + stt (A)
                X = X_t[u]
                if u % APER == APER - 1:
                    lt = ltpool.tile([128, 1024], f32, tag="lt")
                    nc.scalar.activation(lt[:], X[:], AF.Ln, scale=-1.0)
                    w = ltpool.tile([128, 1024], f32, tag="w")
                    nc.vector.scalar_tensor_tensor(
                        w[:], lt[:], -0.5, X[:], ALU.mult, ALU.add
                    )
                    w_t[u] = w
                else:
                    w1 = w1pool.tile([128, 1024], fp16, tag="w1")
                    nc.vector.tensor_scalar(
                        w1[:], X[:].bitcast(i32), -CFAST, K1, ALU.mult, ALU.add
                    )
                    for hh in range(2):
                        nc.tensor.matmul(
                            X[:, hh * 512 : (hh + 1) * 512],
                            eye[:],
                            w1[:, hh * 512 : (hh + 1) * 512],
                            start=False,
                            stop=True,
                        )

            def stage3(u):  # Exp -> g, then S-matmuls
                b, T, h = half_info(u)
                apath = u % APER == APER - 1
                g = gpool.tile([128, 1024], fp16, tag="g")
                g_t[u] = g
                if apath:
                    w = w_t.pop(u)
                    nc.scalar.activation(g[:], w[:], AF.Exp, bias=biasv[:, 1:2], scale=1.0)
                else:
                    X = X_t[u]
                    nc.scalar.activation(g[:], X[:], AF.Exp, bias=biasv[:, 0:1], scale=1.0)
                X_t.pop(u, None)
                q, s = divmod(T, 4)
                key = (b, q)
                if key not in sacc_t:
                    sacc_t[key] = spool.tile([128, 512], f32, tag="sacc", name=f"sa{b}_{q}")
                sacc = sacc_t[key]
                base = 32 * s
                for hh in range(2):
                    k = 2 * h + hh
                    c3 = (b * NK + k) * 3
                    nc.tensor.matmul(
                        sacc[base : base + 3, :],
                        wq[:, c3 : c3 + 3],
                        g[:, hh * 512 : (hh + 1) * 512],
                        start=(k == 0),
                        stop=(k == NK - 1),
                        tile_position=(0, base),
                        skip_group_check=(base != 0),
                    )
                if h == 1 and s == 3:
                    finish_q(b, q)

            def finish_q(b, q):
                sacc = sacc_t.pop((b, q))
                stage = stgpool.tile([128, 512], f32, tag="stage", name=f"st{b}_{q}")
                nc.vector.tensor_copy(stage[:], sacc[:])
                nc.sync.dma_start(out_d[b, q], stage[:])

            STEP = step_ms
            for u in range(NHALF + 2):
                t_it = STEP * u
                if u < NHALF:
                    with tc.tile_wait_until(t_it):
                        stage1(u)
                if 1 <= u <= NHALF:
                    with tc.tile_wait_until(t_it + STEP * 0.33):
                        stage2(u - 1)
                if 2 <= u <= NHALF + 1:
                    with tc.tile_wait_until(t_it + STEP * 0.66):
                        stage3(u - 2)
    nc.compile()
    return nc


def _split3(a, bf):
    h = a.astype(bf)
    m = (a - h.astype(np.float64)).astype(bf)
    l = (a - h.astype(np.float64) - m.astype(np.float64)).astype(bf)
    return h, m, l


def _prep_inputs(vortex_feature, points):
    import ml_dtypes

    bf = ml_dtypes.bfloat16
    vf = np.asarray(vortex_feature, dtype=np.float64)
    pts_full = np.asarray(points, dtype=np.float64)
    y, x, tau = vf[:, :, 0], vf[:, :, 1], vf[:, :, 2]
    sig2 = vf[:, :, 3] ** 2
    a_n = 1.0 / sig2                       # a' = 1/sig^2 (v = a'(r2+eps))
    eps_n = EPS0 + EPS1 * (y * y + x * x)

    # lhsT rows [B, KROWS, N]: NEGATED triple-split entries (X = -v).
    lhst = np.zeros((B, KROWS, N), dtype=bf)
    for b in range(B):
        A3 = _split3(-a_n[b], bf)
        CY3 = _split3(2.0 * a_n[b] * y[b], bf)
        CX3 = _split3(2.0 * a_n[b] * x[b], bf)
        AYY3 = _split3(-a_n[b] * y[b] * y[b], bf)
        AXX3 = _split3(-a_n[b] * x[b] * x[b], bf)
        aeps = (-a_n[b] * eps_n[b]).astype(bf)
        rows = []
        for (uh, um, ul) in (A3, CY3):
            rows += [uh, uh, um, uh, ul, um]
        rows += list(AYY3)
        for (uh, um, ul) in (A3, CX3):
            rows += [uh, uh, um, uh, ul, um]
        rows += list(AXX3)
        rows.append(aeps)
        lhst[b] = np.stack(rows, 0)

    # fp16 S-weights: rows {tau*q, tau*x*q, tau*y*q}, q = exp(a'*eps)*sqrt(a')
    q = np.exp(a_n * eps_n) * np.sqrt(a_n)
    wfull = np.stack([tau * q, tau * x * q, tau * y * q], axis=-1)  # [B, N, 3]
    assert np.abs(wfull).max() < 6.0e4, "fp16 S-weight overflow"
    w3 = wfull.astype(np.float16)
    wq = np.ascontiguousarray(
        w3.reshape(B, NK, 128, 3).transpose(2, 0, 1, 3).reshape(128, B * NK * 3)
    )

    eyem = np.eye(128, dtype=np.float16)
    biasv = np.zeros((128, 2), dtype=np.float32)  # K folded into K1; both biases 0

    in_maps = []
    for i in range(NCORES):
        slp = pts_full[:, i * HPC : (i + 1) * HPC].reshape(B, PPB, 2)
        pts = np.ascontiguousarray(slp.transpose(0, 2, 1))  # [B, 2, PPB]
        rhs = np.zeros((B, KROWS, PPB), dtype=bf)
        for b in range(B):
            py, px = pts[b, 0], pts[b, 1]
            PYY3 = _split3(py * py, bf)
            PY3 = _split3(py, bf)
            PXX3 = _split3(px * px, bf)
            PX3 = _split3(px, bf)
            ones = np.ones(PPB, dtype=bf)
            rows = []
            for (wh_, wm_, wl_) in (PYY3, PY3):
                rows += [wh_, wm_, wh_, wl_, wh_, wm_]
            rows += [ones] * 3
            for (wh_, wm_, wl_) in (PXX3, PX3):
                rows += [wh_, wm_, wh_, wl_, wh_, wm_]
            rows += [ones] * 3
            rows.append(ones)
            rhs[b] = np.stack(rows, 0)
        rhs16 = np.ascontiguousarray(
            rhs.reshape(B, KROWS, 16, PPB // 16).transpose(0, 2, 1, 3)
        )
        in_maps.append(
            {"rhs": rhs16, "lhst": lhst, "wq": wq, "eye": eyem, "biasv": biasv}
        )
    return in_maps


def _assemble(results, points):
    pts_full = np.asarray(points, dtype=np.float64)
    out = np.zeros((B, H, W, 2), dtype=np.float32)
    slot_rows = np.concatenate([np.arange(32 * s, 32 * s + 3) for s in range(4)])
    for i in range(NCORES):
        o = np.asarray(results[i]["out"]).astype(np.float64)  # [B, 4, 128, 512]
        S = o[:, :, slot_rows, :].reshape(B, 4, 4, 3, 512)    # [B, q, s, r, 512]
        S = S.transpose(0, 3, 1, 2, 4).reshape(B, 3, PPB)     # [B, r, PPB]
        slp = pts_full[:, i * HPC : (i + 1) * HPC].reshape(B, PPB, 2)
        py, px = slp[..., 0], slp[..., 1]
        u = px * S[:, 0] - S[:, 1]
        v = S[:, 2] - py * S[:, 0]
        uv = np.stack([u, v], axis=-1).reshape(B, HPC, W, 2)
        out[:, i * HPC : (i + 1) * HPC] = uv.astype(np.float32)
    return out


def _run(vortex_feature, points, trace=False):
    _, _, _, run_bass_kernel_spmd, _b = _bass_modules()
    if "nc" not in _cache:
        _cache["nc"] = _build_nc()
    in_maps = _prep_inputs(vortex_feature, points)
    res = run_bass_kernel_spmd(
        _cache["nc"], in_maps, list(range(NCORES)), trace=trace
    )
    return _assemble(res.results, points), res


def kernel(vortex_feature, points):
    out, _ = _run(vortex_feature, points, trace=False)
    return out
